# Initial kernel scaffold
#
"""AttentionBlock Trainium2 kernel.

Computes, per batch element b (one NeuronCore each, B=8 over 8 cores):
    h   = GroupNorm(x; 8 groups) * gamma + beta
    q,k,v = 1x1 conv(h)
    attn  = softmax(q^T k / sqrt(C), axis=j)
    out   = x + Wo (v @ attn^T) + bo

Layout strategy (per core, x_b: [C=256, L=2048] f32):
  - GroupNorm is folded into the conv weights: h = A*x + B (per-channel) =>
    W' = W diag(A), b' = W B + b. Exact.
  - k-bias is dropped: it adds a per-i constant to logits, softmax-invariant.
  - Scores are computed TRANSPOSED: S^T[j,i] = sum_c k[c,j] q[c,i] via
    matmul(lhsT=k_tile, rhs=q_block). exp(S^T/sqrt(C)) = P^T goes straight to
    SBUF and is used as the *moving* operand of the AV matmul with stationary
    v^T tiles, producing O[c,i] directly -- no probability transposes at all.
  - Row sums r_i = sum_j P[i,j] via a bf16 tree-reduction over the 16 P^T
    tiles (DVE+GpSimd) followed by a single ones-vector matmul per i-chunk.
  - Normalization by 1/r and the residual are applied in the epilogue.
  - Matmuls run as float32r (full PE rate); AV runs in bf16.
"""

import numpy as np
from contextlib import ExitStack

import concourse.bass as bass
import concourse.mybir as mybir
import concourse.tile as tile
from concourse.bass_utils import run_bass_kernel_spmd

C = 256
L = 2048
B = 8
GROUPS = 8
GSIZE = C // GROUPS  # 32
EPS = 1e-5
SCALE = 1.0 / np.sqrt(C)  # 1/16
P = 128  # partitions
NB = L // 512  # 4 free-dim blocks of 512
NT = L // P  # 16 partition tiles of 128

FP32 = mybir.dt.float32
FP32R = mybir.dt.float32r
BF16 = mybir.dt.bfloat16

AF = mybir.ActivationFunctionType
OP = mybir.AluOpType


def r32(ap):
    return ap.bitcast(FP32R)


def build_nc():
    nc = bass.Bass(target_bir_lowering=False)

    x = nc.dram_tensor("x", [C, L], FP32, kind="ExternalInput")
    gamma = nc.dram_tensor("gamma", [C, 1], FP32, kind="ExternalInput")
    beta = nc.dram_tensor("beta", [C, 1], FP32, kind="ExternalInput")
    wq = nc.dram_tensor("wq", [C, C], FP32, kind="ExternalInput")
    wk = nc.dram_tensor("wk", [C, C], FP32, kind="ExternalInput")
    wv = nc.dram_tensor("wv", [C, C], FP32, kind="ExternalInput")
    wo = nc.dram_tensor("wo", [C, C], FP32, kind="ExternalInput")
    bq = nc.dram_tensor("bq", [C, 1], FP32, kind="ExternalInput")
    bv = nc.dram_tensor("bv", [C, 1], FP32, kind="ExternalInput")
    bo = nc.dram_tensor("bo", [C, 1], FP32, kind="ExternalInput")
    # constants
    ident = nc.dram_tensor("ident", [P, P], FP32, kind="ExternalInput")
    ones = nc.dram_tensor("ones", [P, P], FP32, kind="ExternalInput")
    ohcg = nc.dram_tensor("ohcg", [C, GROUPS], FP32, kind="ExternalInput")
    ohgc = nc.dram_tensor("ohgc", [GROUPS, C], FP32, kind="ExternalInput")

    out = nc.dram_tensor("out", [C, L], FP32, kind="ExternalOutput")

    with tile.TileContext(nc) as tc, ExitStack() as ctx:
        per = ctx.enter_context(tc.tile_pool(name="per", bufs=1))
        dbl = ctx.enter_context(tc.tile_pool(name="dbl", bufs=2))

        # ---- persistent SBUF tiles ----
        # consts layout (cols of [128, 1024] f32):
        #   0:128 ident | 128:256 ones | 256:264 ohcg0 | 264:272 ohcg1
        #   [0:8]272:528 ohgc | 528/529 gamma | 530/531 beta | 532/533 bq
        #   534/535 bo | 536/537 bvin | 544:800 bvb
        cst = per.tile([P, 1024], FP32, tag="cst")
        xs = [per.tile([P, L], FP32, tag=f"x{i}") for i in range(2)]
        wt = per.tile([P, 8, C], FP32, tag="wt")  # [cin_p, (w,ic), cout]
        ks = [per.tile([P, L], FP32, tag=f"k{i}") for i in range(2)]
        vt = per.tile([P, NT, C], BF16, tag="vt")  # v^T: [j_p, jt, c]
        pt = per.tile([P, NT, L], BF16, tag="pt")  # P^T: [j_p, jt, i]
        racc = per.tile([P, L], FP32, tag="racc")
        rsb = per.tile([1, L], FP32, tag="rsb")
        rb = per.tile([P, L], FP32, tag="rb")
        gn = per.tile([P, 16], FP32, tag="gn")
        # q tiles; later reused for O (disjoint lifetimes, same tag)
        qs = [dbl.tile([P, L], FP32, tag="qo") for i in range(2)]
        outs = [dbl.tile([P, L], FP32, tag="ob") for i in range(2)]

        IDN = cst[:, 0:128]
        ONES = cst[:, 128:256]
        OHCG = [cst[:, 256:264], cst[:, 264:272]]
        OHGC = cst[0:GROUPS, 272:528]
        GAM = [cst[:, 528:529], cst[:, 529:530]]
        BET = [cst[:, 530:531], cst[:, 531:532]]
        BQ = [cst[:, 532:533], cst[:, 533:534]]
        BON = [cst[:, 534:535], cst[:, 535:536]]
        BVIN = [cst[:, 536:537], cst[:, 537:538]]
        BVB = cst[:, 544:800]

        # gn scratch layout
        A_ = [gn[:, 0:1], gn[:, 1:2]]
        B_ = [gn[:, 2:3], gn[:, 3:4]]
        STATS = gn[0:GROUPS, 4:6]  # mu | rstd
        MU = gn[0:GROUPS, 4:5]
        RSTD = gn[0:GROUPS, 5:6]
        EX2 = gn[0:GROUPS, 6:7]
        VAR = gn[0:GROUPS, 7:8]
        BQF = [gn[:, 8:9], gn[:, 9:10]]  # folded q bias
        BKF = [gn[:, 10:11], gn[:, 11:12]]  # folded k bias
        BVF = [gn[:, 12:13], gn[:, 13:14]]  # folded v bias (column form)
        BVROW = gn[0:1, 8:10]  # unused marker (row form kept in vrow)
        vrow = per.tile([1, C], FP32, tag="vrow")  # folded v bias row form

        # ---- input DMAs ----
        nc.sync.dma_start(out=IDN, in_=ident[:, :])
        nc.sync.dma_start(out=ONES, in_=ones[:, :])
        nc.sync.dma_start(out=OHCG[0], in_=ohcg[0:P, :])
        nc.sync.dma_start(out=OHCG[1], in_=ohcg[P:C, :])
        nc.sync.dma_start(out=OHGC, in_=ohgc[:, :])
        for i in range(2):
            sl = slice(i * P, (i + 1) * P)
            nc.sync.dma_start(out=GAM[i], in_=gamma[sl, :])
            nc.sync.dma_start(out=BET[i], in_=beta[sl, :])
            nc.sync.dma_start(out=BQ[i], in_=bq[sl, :])
            nc.sync.dma_start(out=BON[i], in_=bo[sl, :])
            nc.sync.dma_start(out=BVIN[i], in_=bv[sl, :])
            nc.sync.dma_start(out=xs[i], in_=x[sl, :])

        # raw weights into scratch, transposed into wt
        wsc = ctx.enter_context(tc.tile_pool(name="wsc", bufs=2))

        with tc.tile_pool(name="ps_init", bufs=1) as psi:
            # ---- weight transposes: wt[:, 2*w+ic, oc*128:+128] = W[oc_p, ic_f]^T
            for wi, wdram in enumerate((wq, wk, wv, wo)):
                for oc in range(2):
                    wraw = wsc.tile([P, C], FP32, tag="wraw")
                    nc.sync.dma_start(out=wraw, in_=wdram[oc * P:(oc + 1) * P, :])
                    for ic in range(2):
                        tp = psi.tile([P, P], FP32, tag="wtp", bufs=2)
                        nc.tensor.transpose(
                            out=tp, in_=wraw[:, ic * P:(ic + 1) * P], identity=IDN
                        )
                        nc.vector.tensor_copy(
                            out=wt[:, 2 * wi + ic, oc * P:(oc + 1) * P], in_=tp
                        )

            # ---- GroupNorm stats ----
            sq = psi  # just for namespacing clarity
            for i in range(2):
                ssq = wsc.tile([P, 2], FP32, tag="ssq", bufs=2)
                scr = wsc.tile([P, L], BF16, tag="sqscr", bufs=2)
                nc.vector.tensor_reduce(
                    out=ssq[:, 0:1], in_=xs[i], axis=mybir.AxisListType.X, op=OP.add
                )
                nc.scalar.activation(
                    out=scr, in_=xs[i], func=AF.Square, accum_out=ssq[:, 1:2]
                )
                gsum = psi.tile([GROUPS, 2], FP32, tag="tiny", bufs=2)
                if i == 0:
                    gsum_hold = gsum
                    nc.tensor.matmul(gsum, lhsT=OHCG[0], rhs=ssq, start=True, stop=False)
                else:
                    nc.tensor.matmul(
                        gsum_hold, lhsT=OHCG[1], rhs=ssq, start=False, stop=True
                    )
            NG = float(GSIZE * L)
            nc.scalar.mul(out=MU, in_=gsum_hold[:, 0:1], mul=1.0 / NG)
            nc.scalar.mul(out=EX2, in_=gsum_hold[:, 1:2], mul=1.0 / NG)
            nc.vector.tensor_tensor(out=VAR, in0=MU, in1=MU, op=OP.mult)
            nc.vector.tensor_tensor(out=VAR, in0=EX2, in1=VAR, op=OP.subtract)
            nc.scalar.activation(out=VAR, in_=VAR, func=AF.Sqrt, bias=EPS)
            nc.vector.reciprocal(out=RSTD, in_=VAR)

            # broadcast group stats to channels: bc[c, 0:2] = (mu_c, rstd_c)
            for i in range(2):
                bc = psi.tile([P, 2], FP32, tag="tiny", bufs=2)
                nc.tensor.matmul(
                    bc, lhsT=OHGC[:, i * P:(i + 1) * P], rhs=STATS, start=True, stop=True
                )
                # A = rstd*gamma ; B = beta - mu*A
                nc.vector.tensor_tensor(out=A_[i], in0=bc[:, 1:2], in1=GAM[i], op=OP.mult)
                nc.vector.tensor_tensor(out=B_[i], in0=bc[:, 0:1], in1=A_[i], op=OP.mult)
                nc.vector.tensor_tensor(out=B_[i], in0=BET[i], in1=B_[i], op=OP.subtract)

            # fold GroupNorm scale into weights: wt[cin, :, :] *= A[cin]
            for wi in range(4):
                for ic in range(2):
                    nc.vector.tensor_scalar_mul(
                        out=wt[:, 2 * wi + ic, :], in0=wt[:, 2 * wi + ic, :],
                        scalar1=A_[ic],
                    )

            # folded biases: b' = W' B + b  (per out-channel, column form)
            for wi, badd, dst in ((0, BQ, BQF), (1, None, BKF), (2, BVIN, BVF)):
                for oc in range(2):
                    bp = psi.tile([P, 1], FP32, tag="tiny2", bufs=2)
                    for ic in range(2):
                        nc.tensor.matmul(
                            bp, lhsT=wt[:, 2 * wi + ic, oc * P:(oc + 1) * P],
                            rhs=B_[ic], start=(ic == 0), stop=(ic == 1),
                        )
                    if badd is not None:
                        nc.vector.tensor_tensor(
                            out=dst[oc], in0=bp, in1=badd[oc], op=OP.add
                        )
                    else:
                        nc.vector.tensor_copy(out=dst[oc], in_=bp)

            # v bias to row form: vrow[0, oc*128:+128] = BVF[oc]^T
            for oc in range(2):
                tpb = psi.tile([P, P], FP32, tag="wtp", bufs=2)
                nc.tensor.transpose(
                    out=tpb[0:1, :], in_=BVF[oc], identity=IDN
                )
                nc.vector.tensor_copy(
                    out=vrow[:, oc * P:(oc + 1) * P], in_=tpb[0:1, :]
                )

        # ---- projections ----
        with tc.tile_pool(name="ps_proj", bufs=1) as psp:
            # bvb broadcast: [128, 256] = ones_col @ vrow
            bvp = psp.tile([P, C], FP32, tag="bvb", bufs=1)
            nc.tensor.matmul(
                bvp, lhsT=r32(ONES[0:1, :]), rhs=r32(vrow[:, :]), start=True, stop=True
            )
            nc.scalar.copy(out=BVB, in_=bvp)

            # q, k: [o_p, l] = W'^T.T @ x
            for wi, dsts, badd in ((0, qs, BQF), (1, ks, BKF)):
                for oc in range(2):
                    for nb in range(NB):
                        pp = psp.tile([P, 512], FP32, tag="qkp", bufs=4)
                        for ic in range(2):
                            nc.tensor.matmul(
                                pp,
                                lhsT=r32(wt[:, 2 * wi + ic, oc * P:(oc + 1) * P]),
                                rhs=r32(xs[ic][:, nb * 512:(nb + 1) * 512]),
                                start=(ic == 0), stop=(ic == 1),
                            )
                        nc.vector.tensor_scalar_add(
                            out=dsts[oc][:, nb * 512:(nb + 1) * 512],
                            in0=pp, scalar1=badd[oc],
                        )

            # v^T: [l_p, c] per l-tile
            for lt in range(NT):
                vp = psp.tile([P, C], FP32, tag="vp", bufs=2)
                for ic in range(2):
                    nc.tensor.matmul(
                        vp,
                        lhsT=r32(xs[ic][:, lt * P:(lt + 1) * P]),
                        rhs=r32(wt[:, 4 + ic, :]),
                        start=(ic == 0), stop=(ic == 1),
                    )
                nc.vector.tensor_tensor(
                    out=vt[:, lt, :], in0=vp, in1=BVB, op=OP.add
                )

        # ---- scores + exp ----
        with tc.tile_pool(name="ps_attn", bufs=1) as psa:
            for jt in range(NT):
                sp = psa.tile([P, L], FP32, tag="stp", bufs=2)
                for nb in range(NB):
                    for ic in range(2):
                        nc.tensor.matmul(
                            sp[:, nb * 512:(nb + 1) * 512],
                            lhsT=r32(ks[ic][:, jt * P:(jt + 1) * P]),
                            rhs=r32(qs[ic][:, nb * 512:(nb + 1) * 512]),
                            start=(ic == 0), stop=(ic == 1),
                        )
                nc.scalar.activation(
                    out=pt[:, jt, :], in_=sp, func=AF.Exp, scale=float(SCALE)
                )

        # ---- row sums: bf16 tree over jt tiles, alternating DVE/GpSimd ----
        tr = ctx.enter_context(tc.tile_pool(name="tr", bufs=8))
        lvl = [pt[:, j, :] for j in range(NT)]
        eng = [nc.vector, nc.gpsimd]
        li = 0
        while len(lvl) > 2:
            nxt = []
            for t in range(len(lvl) // 2):
                dst = tr.tile([P, L], BF16, tag=f"tr{li}_{t}", bufs=1)
                eng[t % 2].tensor_tensor(
                    out=dst, in0=lvl[2 * t], in1=lvl[2 * t + 1], op=OP.add
                )
                nxt.append(dst)
            lvl = nxt
            li += 1
        nc.vector.tensor_tensor(out=racc, in0=lvl[0], in1=lvl[1], op=OP.add)

        # ---- AV + normalization prep ----
        os_ = [qs[0], qs[1]]  # reuse q slots for O[c, i]
        with tc.tile_pool(name="ps_av", bufs=1) as psv:
            for nb in range(NB):
                rp = psv.tile([1, 512], FP32, tag="rp", bufs=2)
                nc.tensor.matmul(
                    rp, lhsT=r32(ONES[:, 0:1]),
                    rhs=r32(racc[:, nb * 512:(nb + 1) * 512]),
                    start=True, stop=True,
                )
                nc.vector.tensor_copy(out=rsb[:, nb * 512:(nb + 1) * 512], in_=rp)
                rbp = psv.tile([P, 512], FP32, tag="rbp", bufs=2)
                nc.tensor.matmul(
                    rbp, lhsT=r32(ONES[0:1, :]),
                    rhs=r32(rsb[:, nb * 512:(nb + 1) * 512]),
                    start=True, stop=True,
                )
                nc.vector.reciprocal(out=rb[:, nb * 512:(nb + 1) * 512], in_=rbp)

            for cc in range(2):
                for nb in range(NB):
                    op_ = psv.tile([P, 512], FP32, tag="op", bufs=3)
                    for jt in range(NT):
                        nc.tensor.matmul(
                            op_,
                            lhsT=vt[:, jt, cc * P:(cc + 1) * P],
                            rhs=pt[:, jt, nb * 512:(nb + 1) * 512],
                            start=(jt == 0), stop=(jt == NT - 1),
                        )
                    nc.vector.tensor_tensor(
                        out=os_[cc][:, nb * 512:(nb + 1) * 512],
                        in0=op_, in1=rb[:, nb * 512:(nb + 1) * 512], op=OP.mult,
                    )

        # ---- final projection + epilogue ----
        with tc.tile_pool(name="ps_fin", bufs=1) as psf:
            for oc in range(2):
                for nb in range(NB):
                    fp = psf.tile([P, 512], FP32, tag="fp", bufs=4)
                    for cc in range(2):
                        nc.tensor.matmul(
                            fp,
                            lhsT=r32(wt[:, 6 + cc, oc * P:(oc + 1) * P]),
                            rhs=r32(os_[cc][:, nb * 512:(nb + 1) * 512]),
                            start=(cc == 0), stop=(cc == 1),
                        )
                    tsb = wsc.tile([P, 512], FP32, tag="tsb", bufs=3)
                    nc.scalar.activation(
                        out=tsb, in_=fp, func=AF.Identity, bias=BON[oc], scale=1.0
                    )
                    nc.vector.tensor_tensor(
                        out=outs[oc][:, nb * 512:(nb + 1) * 512],
                        in0=tsb, in1=xs[oc][:, nb * 512:(nb + 1) * 512], op=OP.add,
                    )
                    nc.sync.dma_start(
                        out=out[oc * P:(oc + 1) * P, nb * 512:(nb + 1) * 512],
                        in_=outs[oc][:, nb * 512:(nb + 1) * 512],
                    )

    nc.compile()
    return nc


def make_in_maps(inputs):
    x = np.ascontiguousarray(np.asarray(inputs["x"], dtype=np.float32))
    assert x.shape == (B, C, L), x.shape
    f32 = lambda a: np.ascontiguousarray(np.asarray(a, dtype=np.float32))
    ohcg = np.zeros((C, GROUPS), np.float32)
    for c in range(C):
        ohcg[c, c // GSIZE] = 1.0
    base = {
        "gamma": f32(inputs["gamma"]).reshape(C, 1),
        "beta": f32(inputs["beta"]).reshape(C, 1),
        "wq": f32(inputs["Wq"]),
        "wk": f32(inputs["Wk"]),
        "wv": f32(inputs["Wv"]),
        "wo": f32(inputs["Wo"]),
        "bq": f32(inputs["bq"]).reshape(C, 1),
        "bv": f32(inputs["bv"]).reshape(C, 1),
        "bo": f32(inputs["bo"]).reshape(C, 1),
        "ident": np.eye(P, dtype=np.float32),
        "ones": np.ones((P, P), np.float32),
        "ohcg": ohcg,
        "ohgc": np.ascontiguousarray(ohcg.T),
    }
    return [dict(base, x=x[b]) for b in range(B)]


def kernel(**inputs):
    nc = build_nc()
    in_maps = make_in_maps(inputs)
    res = run_bass_kernel_spmd(nc, in_maps, core_ids=list(range(B)))
    return np.stack([r["out"] for r in res.results], axis=0)


if __name__ == "__main__":
    rng = np.random.default_rng(0)
    ins = {
        "x": rng.standard_normal((B, C, L), dtype=np.float32),
        "gamma": np.ones(C, np.float32),
        "beta": np.zeros(C, np.float32),
    }
    for n in ("q", "k", "v", "o"):
        ins["W" + n] = rng.uniform(-1 / 16, 1 / 16, (C, C)).astype(np.float32)
        ins["b" + n] = rng.uniform(-1 / 16, 1 / 16, (C,)).astype(np.float32)
    out = kernel(**ins)
    print(out.shape, out.dtype)


# revision 20
# speedup vs baseline: 1.1865x; 1.1865x over previous
"""AttentionBlock Trainium2 kernel (B=8 data-parallel over 8 NeuronCores).

Per batch element b (one core each), x_b: [C=256, L=2048] f32:
    h   = GroupNorm(x; 8 groups) * gamma + beta
    q,k,v = 1x1 conv(h);  attn = softmax(q^T k / sqrt(C), axis=j)
    out = x + Wo (v @ attn^T) + bo

Structure:
  - k-bias dropped: adds a per-query constant to logits -> softmax-invariant.
  - Scores computed TRANSPOSED: S^T[j,i] = sum_c k[c,j] q[c,i] via
    matmul(lhsT=k_tile, rhs=q_block). P^T = exp(S^T * scale) lands in SBUF
    and is the *moving* operand of the AV matmul with stationary v^T tiles
    -> O[c,i] directly. No probability transposes anywhere.
  - Row sums r_i = sum_j P[i,j]: two accumulation chains over the 16 P^T
    tiles (DVE + GpSimd), then ones-vector matmuls; 1/r broadcast across
    partitions with a K=1 ones matmul.
  - MODE selects precision of the attention path:
      f32   : float32r matmuls (PE single-pass fp32), bf16 P/V.
      av8   : P and V in fp8e4, AV matmul in DoubleRow (2 K-rows/cycle).
      full8 : h/q/k/P/V/O in fp8e4, all big matmuls DoubleRow.
    The residual, GroupNorm statistics and epilogue stay exact fp32.
"""

import os

import numpy as np
from contextlib import ExitStack

import concourse.bass as bass
import concourse.bacc as bacc
import concourse.mybir as mybir
import concourse.tile as tile
from concourse.bass_utils import run_bass_kernel_spmd

C = 256
L = 2048
B = 8
GROUPS = 8
GSIZE = C // GROUPS  # 32
EPS = 1e-5
SCALE = 1.0 / np.sqrt(C)  # 1/16
P = 128
NB = L // 512  # 4
NT = L // P  # 16

FP32 = mybir.dt.float32
FP32R = mybir.dt.float32r
BF16 = mybir.dt.bfloat16
FP8 = mybir.dt.float8e4

AF = mybir.ActivationFunctionType
OP = mybir.AluOpType
DR = mybir.MatmulPerfMode.DoubleRow

MODE = os.environ.get("ATT_MODE", "f32")


def r32(ap):
    return ap.bitcast(FP32R)


def build_nc(stop_after="full", mode=None):
    mode = mode or MODE
    assert mode in ("f32", "av8", "full8")
    fp8_av = mode in ("av8", "full8")
    fp8_all = mode == "full8"

    nc = bacc.Bacc("TRN2", target_bir_lowering=False)

    x = nc.dram_tensor("x", [C, L], FP32, kind="ExternalInput")
    gamma = nc.dram_tensor("gamma", [C, 1], FP32, kind="ExternalInput")
    beta = nc.dram_tensor("beta", [C, 1], FP32, kind="ExternalInput")
    wd = {}
    for n in ("wq", "wk", "wv", "wo"):
        wd[n] = nc.dram_tensor(n, [C, C], FP32, kind="ExternalInput")
    bq = nc.dram_tensor("bq", [C, 1], FP32, kind="ExternalInput")
    bv = nc.dram_tensor("bv", [C, 1], FP32, kind="ExternalInput")
    bo = nc.dram_tensor("bo", [C, 1], FP32, kind="ExternalInput")
    ident = nc.dram_tensor("ident", [P, P], FP32, kind="ExternalInput")
    ones = nc.dram_tensor("ones", [P, P], FP32, kind="ExternalInput")
    ohcg = nc.dram_tensor("ohcg", [C, GROUPS], FP32, kind="ExternalInput")
    ohgc = nc.dram_tensor("ohgc", [GROUPS, C], FP32, kind="ExternalInput")
    out = nc.dram_tensor("out", [C, L], FP32, kind="ExternalOutput")

    with tile.TileContext(nc) as tc, ExitStack() as ctx:
        _body(nc, tc, ctx, locals(), stop_after, fp8_av, fp8_all)
    nc.compile()
    return nc


def _body(nc, tc, ctx, env, stop_after, fp8_av, fp8_all):
    x, gamma, beta, bq, bv, bo = (
        env["x"], env["gamma"], env["beta"], env["bq"], env["bv"], env["bo"]
    )
    wd, ident, ones, ohcg, ohgc, out = (
        env["wd"], env["ident"], env["ones"], env["ohcg"], env["ohgc"], env["out"]
    )
    pt_dt = FP8 if fp8_av else BF16
    qk_dt = FP8 if fp8_all else FP32  # f32r-written when FP32
    h_dt = FP8 if fp8_all else FP32
    wt_dt = FP8 if fp8_all else FP32

    per = ctx.enter_context(tc.tile_pool(name="per", bufs=1))

    cst = per.tile([P, 1024], FP32, tag="cst", name="cst")
    xs = [per.tile([P, L], FP32, tag=f"x{i}", name=f"x{i}") for i in range(2)]
    wt = per.tile([P, 8, C], wt_dt, tag="wt", name="wt")  # [cin_p, (w,ic), cout]
    qt = per.tile([P, 2, L], qk_dt, tag="qt", name="qt")  # [c_p, cc, i]
    kt = per.tile([P, 2, L], qk_dt, tag="kt", name="kt")
    vt = per.tile([P, NT, C], pt_dt, tag="vt", name="vt")  # v^T: [j_p, jt, c]
    pt = per.tile([P, NT, L], pt_dt, tag="pt", name="pt")  # P^T: [j_p, jt, i]
    # O[c, i]: full8 gets its own fp8 tile; otherwise reuse qt (disjoint life)
    if fp8_all:
        ot = per.tile([P, 2, L], FP8, tag="ot", name="ot")
    else:
        ot = qt
    rsb = per.tile([1, L], FP32, tag="rsb", name="rsb")  # 1/r
    gn = per.tile([P, 24], FP32, tag="gn", name="gn")
    onesb = per.tile([P, 1], BF16, tag="onesb", name="onesb")
    ones8 = per.tile([P, 2, 32], FP8, tag="ones8", name="ones8")
    onesr = per.tile([1, P], FP32R, tag="onesr", name="onesr")
    outs = [per.tile([P, L], FP32, tag=f"ob{i}", name=f"ob{i}") for i in range(2)]

    IDN = cst[:, 0:128]
    OHCG = [cst[:, 256:264], cst[:, 264:272]]
    OHGC = cst[0:GROUPS, 272:528]
    GAM = [cst[:, 528:529], cst[:, 529:530]]
    BET = [cst[:, 530:531], cst[:, 531:532]]
    BQ = [cst[:, 532:533], cst[:, 533:534]]
    BON = [cst[:, 534:535], cst[:, 535:536]]
    BVB = cst[:, 544:800]

    A_ = [gn[:, 0:1], gn[:, 1:2]]
    B_ = [gn[:, 2:3], gn[:, 3:4]]
    STATS = gn[0:GROUPS, 4:6]
    MU = gn[0:GROUPS, 4:5]
    RSTD = gn[0:GROUPS, 5:6]
    EX2 = gn[0:GROUPS, 6:7]
    VAR = gn[0:GROUPS, 7:8]
    EPSAP = gn[0:GROUPS, 15:16]
    SSQ = [gn[:, 16:18], gn[:, 18:20]]

    nc.vector.memset(onesb, 1.0)
    nc.vector.memset(ones8, 1.0)
    nc.vector.memset(EPSAP, EPS)

    # ---- input DMAs (x first: it is the critical path) ----
    for i in range(2):
        nc.sync.dma_start(out=xs[i], in_=x[i * P:(i + 1) * P, :])
    nc.sync.dma_start(out=IDN, in_=ident[:, :])
    nc.sync.dma_start(out=onesr, in_=r32(ones[0:1, :]))
    nc.sync.dma_start(out=OHCG[0], in_=ohcg[0:P, :])
    nc.sync.dma_start(out=OHCG[1], in_=ohcg[P:C, :])
    nc.sync.dma_start(out=OHGC, in_=ohgc[:, :])
    for i in range(2):
        sl = slice(i * P, (i + 1) * P)
        nc.sync.dma_start(out=GAM[i], in_=gamma[sl, :])
        nc.sync.dma_start(out=BET[i], in_=beta[sl, :])
        nc.sync.dma_start(out=BQ[i], in_=bq[sl, :])
        nc.sync.dma_start(out=BON[i], in_=bo[sl, :])
    # bv broadcast across partitions: [128, C] (gpsimd DMA can broadcast)
    bvb_src = bv[:, :].rearrange("c one -> one c").to_broadcast((P, C))
    nc.gpsimd.dma_start(out=BVB, in_=bvb_src)

    def wslice(wi, ic, oc=None):
        s = wt[:, 2 * wi + ic, :] if oc is None else (
            wt[:, 2 * wi + ic, oc * P:(oc + 1) * P])
        return s if fp8_all else r32(s)

    def wpair(wi, oc=None):
        return (wt[:, 2 * wi:2 * wi + 2, :] if oc is None else
                wt[:, 2 * wi:2 * wi + 2, oc * P:(oc + 1) * P])

    # ================= init: weight transposes + GroupNorm stats ==========
    with tc.tile_pool(name="early", bufs=1) as early, \
         tc.tile_pool(name="ps_init", bufs=1, space="PSUM") as psi:
        for wi, wn in enumerate(("wq", "wk", "wv", "wo")):
            for oc in range(2):
                wraw = early.tile([P, C], FP32, tag="wraw", bufs=2, name="wraw")
                nc.sync.dma_start(out=wraw, in_=wd[wn][oc * P:(oc + 1) * P, :])
                for ic in range(2):
                    tp = psi.tile([P, P], FP32, tag="wtp", bufs=2, name="wtp")
                    nc.tensor.transpose(
                        out=tp, in_=wraw[:, ic * P:(ic + 1) * P], identity=IDN
                    )
                    dst = wt[:, 2 * wi + ic, oc * P:(oc + 1) * P]
                    nc.vector.tensor_copy(
                        out=dst if fp8_all else r32(dst), in_=tp
                    )

        gsum = psi.tile([GROUPS, 2], FP32, tag="tiny", bufs=2, name="tiny")
        for i in range(2):
            scr = early.tile([P, L], BF16, tag="sqscr", bufs=1, name="sqscr")
            nc.vector.tensor_reduce(
                out=SSQ[i][:, 0:1], in_=xs[i], axis=mybir.AxisListType.X, op=OP.add
            )
            nc.scalar.activation(
                out=scr, in_=xs[i], func=AF.Square, accum_out=SSQ[i][:, 1:2]
            )
            nc.tensor.matmul(
                gsum, lhsT=OHCG[i], rhs=SSQ[i], start=(i == 0), stop=(i == 1)
            )
        NG = float(GSIZE * L)
        nc.scalar.mul(out=MU, in_=gsum[:, 0:1], mul=1.0 / NG)
        nc.scalar.mul(out=EX2, in_=gsum[:, 1:2], mul=1.0 / NG)
        nc.vector.tensor_tensor(out=VAR, in0=MU, in1=MU, op=OP.mult)
        nc.vector.tensor_tensor(out=VAR, in0=EX2, in1=VAR, op=OP.subtract)
        nc.scalar.activation(out=VAR, in_=VAR, func=AF.Sqrt, bias=EPSAP)
        nc.vector.reciprocal(out=RSTD, in_=VAR)
        for i in range(2):
            bc = psi.tile([P, 2], FP32, tag="tiny", bufs=2, name="tiny")
            nc.tensor.matmul(
                bc, lhsT=OHGC[:, i * P:(i + 1) * P], rhs=STATS,
                start=True, stop=True,
            )
            nc.vector.tensor_tensor(out=A_[i], in0=bc[:, 1:2], in1=GAM[i], op=OP.mult)
            nc.vector.tensor_tensor(out=B_[i], in0=bc[:, 0:1], in1=A_[i], op=OP.mult)
            nc.vector.tensor_tensor(out=B_[i], in0=BET[i], in1=B_[i], op=OP.subtract)

    if stop_after == "init":
        for oc in range(2):
            nc.vector.tensor_copy(out=outs[oc], in_=xs[oc])
            nc.sync.dma_start(out=out[oc * P:(oc + 1) * P, :], in_=outs[oc])
        return

    # ================= h + projections =================
    with tc.tile_pool(name="ph", bufs=1) as ph, \
         tc.tile_pool(name="ps_proj", bufs=1, space="PSUM") as psp:
        ht = ph.tile([P, 2, L], h_dt, tag="ht", name="ht")
        for cc in range(2):
            hdst = ht[:, cc, :]
            nc.scalar.activation(
                out=hdst if fp8_all else r32(hdst), in_=xs[cc],
                func=AF.Identity, scale=A_[cc], bias=B_[cc],
            )

        def hsl(ic, sl):
            s = ht[:, ic, sl]
            return s if fp8_all else r32(s)

        # q, k: [c_p(oc), i]
        for wi, dst, badd in ((0, qt, BQ), (1, kt, None)):
            for oc in range(2):
                for nb in range(NB):
                    sl = slice(nb * 512, (nb + 1) * 512)
                    pp = psp.tile([P, 512], FP32, tag="qkp", bufs=4, name="qkp")
                    if fp8_all:
                        nc.tensor.matmul(
                            pp, lhsT=wpair(wi, oc), rhs=ht[:, :, sl],
                            start=True, stop=True, perf_mode=DR,
                        )
                    else:
                        for ic in range(2):
                            nc.tensor.matmul(
                                pp, lhsT=wslice(wi, ic, oc), rhs=hsl(ic, sl),
                                start=(ic == 0), stop=(ic == 1),
                            )
                    d = dst[:, oc, sl]
                    if not fp8_all:
                        d = r32(d)
                    if badd is not None:
                        nc.vector.tensor_scalar_add(out=d, in0=pp, scalar1=badd[oc])
                    else:
                        nc.vector.tensor_copy(out=d, in_=pp)

        # v^T: [l_p, c]
        for lt in range(NT):
            lsl = slice(lt * P, (lt + 1) * P)
            vp = psp.tile([P, C], FP32, tag="vp", bufs=2, name="vp")
            if fp8_all:
                nc.tensor.matmul(
                    vp, lhsT=ht[:, :, lsl], rhs=wpair(2),
                    start=True, stop=True, perf_mode=DR,
                )
            else:
                for ic in range(2):
                    nc.tensor.matmul(
                        vp, lhsT=hsl(ic, lsl), rhs=wslice(2, ic),
                        start=(ic == 0), stop=(ic == 1),
                    )
            nc.vector.tensor_tensor(out=vt[:, lt, :], in0=vp, in1=BVB, op=OP.add)

    if stop_after == "proj":
        for oc in range(2):
            nc.vector.tensor_copy(out=outs[oc], in_=qt[:, oc, :])
            nc.sync.dma_start(out=out[oc * P:(oc + 1) * P, :], in_=outs[oc])
        return

    # ================= scores + exp + row-sum chains =================
    with tc.tile_pool(name="mid", bufs=1) as mid:
        acc = None
        if not fp8_av:
            acc = [mid.tile([P, L], BF16, tag=f"acc{i}", name=f"acc{i}")
                   for i in range(2)]
        with tc.tile_pool(name="ps_attn", bufs=1, space="PSUM") as psa:
            for jt in range(NT):
                jsl = slice(jt * P, (jt + 1) * P)
                sp = psa.tile([P, L], FP32, tag="stp", bufs=2, name="stp")
                for nb in range(NB):
                    sl = slice(nb * 512, (nb + 1) * 512)
                    if fp8_all:
                        nc.tensor.matmul(
                            sp[:, sl], lhsT=kt[:, :, jsl], rhs=qt[:, :, sl],
                            start=True, stop=True, perf_mode=DR,
                        )
                    else:
                        for ic in range(2):
                            nc.tensor.matmul(
                                sp[:, sl],
                                lhsT=r32(kt[:, ic, jsl]), rhs=r32(qt[:, ic, sl]),
                                start=(ic == 0), stop=(ic == 1),
                            )
                nc.scalar.activation(
                    out=pt[:, jt, :], in_=sp, func=AF.Exp, scale=float(SCALE)
                )
                if not fp8_av:
                    eng = nc.vector if jt < 8 else nc.gpsimd
                    ci = 0 if jt < 8 else 1
                    if jt % 8 == 0:
                        eng.tensor_copy(out=acc[ci], in_=pt[:, jt, :])
                    else:
                        eng.tensor_tensor(
                            out=acc[ci], in0=acc[ci], in1=pt[:, jt, :], op=OP.add
                        )

        if stop_after == "attn":
            for oc in range(2):
                nc.vector.tensor_copy(out=outs[oc], in_=pt[:, oc, :])
                nc.sync.dma_start(out=out[oc * P:(oc + 1) * P, :], in_=outs[oc])
            return

        # ---- r and 1/r broadcast ----
        rbsb = [mid.tile([P, 512], FP32, tag=f"rb{nb}", name=f"rb{nb}")
                for nb in range(NB)]
        with tc.tile_pool(name="ps_r", bufs=1, space="PSUM") as psr:
            for nb in range(NB):
                sl = slice(nb * 512, (nb + 1) * 512)
                if fp8_av:
                    rp = psr.tile([32, 512], FP32, tag="rp", bufs=2, name="rp")
                    for t in range(NT // 2):
                        nc.tensor.matmul(
                            rp, lhsT=ones8, rhs=pt[:, 2 * t:2 * t + 2, sl],
                            start=(t == 0), stop=(t == NT // 2 - 1),
                            perf_mode=DR,
                        )
                else:
                    rp = psr.tile([1, 512], FP32, tag="rp", bufs=2, name="rp")
                    for ci in range(2):
                        nc.tensor.matmul(
                            rp, lhsT=onesb, rhs=acc[ci][:, sl],
                            start=(ci == 0), stop=(ci == 1),
                        )
                with nc.allow_low_precision(reason="f32r rounding"):
                    nc.vector.reciprocal(out=r32(rsb[:, sl]), in_=rp[0:1, :])
                rbp = psr.tile([P, 512], FP32, tag="rbp", bufs=2, name="rbp")
                nc.tensor.matmul(
                    rbp, lhsT=onesr, rhs=r32(rsb[:, sl]), start=True, stop=True
                )
                nc.vector.tensor_copy(out=rbsb[nb], in_=rbp)

        # ================= AV =================
        with tc.tile_pool(name="ps_av", bufs=1, space="PSUM") as psv:
            for cc in range(2):
                csl = slice(cc * P, (cc + 1) * P)
                for nb in range(NB):
                    sl = slice(nb * 512, (nb + 1) * 512)
                    op_ = psv.tile([P, 512], FP32, tag="op", bufs=3, name="op")
                    if fp8_av:
                        for t in range(NT // 2):
                            nc.tensor.matmul(
                                op_,
                                lhsT=vt[:, 2 * t:2 * t + 2, csl],
                                rhs=pt[:, 2 * t:2 * t + 2, sl],
                                start=(t == 0), stop=(t == NT // 2 - 1),
                                perf_mode=DR,
                            )
                    else:
                        for jt in range(NT):
                            nc.tensor.matmul(
                                op_,
                                lhsT=vt[:, jt, csl], rhs=pt[:, jt, sl],
                                start=(jt == 0), stop=(jt == NT - 1),
                            )
                    d = ot[:, cc, sl]
                    if not fp8_all:
                        d = r32(d)
                    nc.vector.tensor_tensor(
                        out=d, in0=op_, in1=rbsb[nb], op=OP.mult
                    )

    if stop_after == "av":
        for oc in range(2):
            nc.vector.tensor_copy(out=outs[oc], in_=ot[:, oc, :])
            nc.sync.dma_start(out=out[oc * P:(oc + 1) * P, :], in_=outs[oc])
        return

    # ================= final projection + epilogue =================
    with tc.tile_pool(name="fin", bufs=1) as fin, \
         tc.tile_pool(name="ps_fin", bufs=1, space="PSUM") as psf:
        for oc in range(2):
            for nb in range(NB):
                sl = slice(nb * 512, (nb + 1) * 512)
                fp = psf.tile([P, 512], FP32, tag="fp", bufs=4, name="fp")
                if fp8_all:
                    nc.tensor.matmul(
                        fp, lhsT=wpair(3, oc), rhs=ot[:, :, sl],
                        start=True, stop=True, perf_mode=DR,
                    )
                else:
                    for cc in range(2):
                        nc.tensor.matmul(
                            fp, lhsT=wslice(3, cc, oc), rhs=r32(ot[:, cc, sl]),
                            start=(cc == 0), stop=(cc == 1),
                        )
                tsb = fin.tile([P, 512], FP32, tag="tsb", bufs=2, name="tsb")
                nc.scalar.activation(
                    out=tsb, in_=fp, func=AF.Identity, bias=BON[oc], scale=1.0
                )
                nc.vector.tensor_tensor(
                    out=outs[oc][:, sl], in0=tsb, in1=xs[oc][:, sl], op=OP.add
                )
                nc.sync.dma_start(
                    out=out[oc * P:(oc + 1) * P, nb * 512:(nb + 1) * 512],
                    in_=outs[oc][:, sl],
                )


def make_in_maps(inputs):
    x = np.ascontiguousarray(np.asarray(inputs["x"], dtype=np.float32))
    assert x.shape == (B, C, L), x.shape
    f32 = lambda a: np.ascontiguousarray(np.asarray(a, dtype=np.float32))
    ohcg = np.zeros((C, GROUPS), np.float32)
    for c in range(C):
        ohcg[c, c // GSIZE] = 1.0
    base = {
        "gamma": f32(inputs["gamma"]).reshape(C, 1),
        "beta": f32(inputs["beta"]).reshape(C, 1),
        "wq": f32(inputs["Wq"]),
        "wk": f32(inputs["Wk"]),
        "wv": f32(inputs["Wv"]),
        "wo": f32(inputs["Wo"]),
        "bq": f32(inputs["bq"]).reshape(C, 1),
        "bv": f32(inputs["bv"]).reshape(C, 1),
        "bo": f32(inputs["bo"]).reshape(C, 1),
        "ident": np.eye(P, dtype=np.float32),
        "ones": np.ones((P, P), np.float32),
        "ohcg": ohcg,
        "ohgc": np.ascontiguousarray(ohcg.T),
    }
    return [dict(base, x=x[b]) for b in range(B)]


_NC_CACHE = {}


def _axon_reset():
    # Un-wedge the axon terminal if a prior run crashed it; benign when healthy.
    try:
        import ctypes
        import jax

        jax.devices()
        ctypes.CDLL("/opt/axon/libaxon_pjrt.so").axon_reset()
    except Exception:
        pass


def kernel(**inputs):
    if MODE not in _NC_CACHE:
        _NC_CACHE[MODE] = build_nc()
    nc = _NC_CACHE[MODE]
    in_maps = make_in_maps(inputs)
    try:
        res = run_bass_kernel_spmd(nc, in_maps, core_ids=list(range(B)))
    except Exception:
        _axon_reset()
        res = run_bass_kernel_spmd(nc, in_maps, core_ids=list(range(B)))
    return np.stack([r["out"] for r in res.results], axis=0)


if __name__ == "__main__":
    rng = np.random.default_rng(0)
    ins = {
        "x": rng.standard_normal((B, C, L), dtype=np.float32),
        "gamma": np.ones(C, np.float32),
        "beta": np.zeros(C, np.float32),
    }
    for n in ("q", "k", "v", "o"):
        ins["W" + n] = rng.uniform(-1 / 16, 1 / 16, (C, C)).astype(np.float32)
        ins["b" + n] = rng.uniform(-1 / 16, 1 / 16, (C,)).astype(np.float32)
    out = kernel(**ins)
    print(out.shape, out.dtype)
